# revision 1
# baseline (speedup 1.0000x reference)
"""Trainium2 Bass kernel for DETR-style greedy-matching GIoU loss.

Self-contained: accepts FULL inputs (B=32), shards batch across 8 NeuronCores
(4 examples per core), runs one SPMD Bass program, sums partial losses on host.

Per-core program (4 examples, lockstep):
  Phase A: DMA gt coords; DMA-broadcast pr coordinate planes [128, 4096].
  Phase B: IoU matrix in gt-major layout [128 gt, 4096 pr] per (example, half);
           top-8 rows per gt column via max8/max_index.
  Phase C: restripe candidates into matching layout
           [128 partitions = 4 ex x 32, free = 8 cols x 8 levels].
  Phase D: 256 exact greedy steps (eager row-kill; cross-partition argmax via
           32x32 stream transposes; broadcasts via small PE matmuls).
  Phase E: gather matched pr boxes (gpsimd ap_gather), compute GIoU loss
           terms, reduce per example via PE ones-matmul.

Exactness: the greedy matching equals the reference's sequential
argmax/mask scan as long as no gt column needs a candidate deeper than its
top-8 (validated offline: max depth seen is 3) and no selected value is <= 0.
The kernel tracks min selected value per example; host falls back to a numpy
replica for flagged examples (never fires in practice).
"""

import numpy as np

B, N, M = 32, 4096, 256
NCORES = 8
EPB = B // NCORES  # 4 examples per core
R = 8              # candidate levels per gt column
EPS = 1e-7
NEG_BIG = -1.0e30

_CACHE = {}
_DISABLE_GATHER = False


# ---------------------------------------------------------------------------
# Bass program
# ---------------------------------------------------------------------------

def _build_program():
    import concourse.bass as bass  # noqa: F401
    import concourse.bacc as bacc
    import concourse.mybir as mybir
    import concourse.tile as tile

    f32 = mybir.dt.float32
    Alu = mybir.AluOpType
    Ax = mybir.AxisListType

    nc = bacc.Bacc("TRN2", target_bir_lowering=False, debug=False)
    pr_d = nc.dram_tensor("pr_boxes", [EPB, N, 4], f32, kind="ExternalInput")
    gt_d = nc.dram_tensor("gt_boxes", [EPB, M, 4], f32, kind="ExternalInput")
    outv_d = nc.dram_tensor("outv", [EPB, 8], f32, kind="ExternalOutput")
    mm_d = nc.dram_tensor("mm", [128, R * 8], f32, kind="ExternalOutput")

    # constants
    ohcomb = np.zeros((128, 128), dtype=np.float32)  # block-diag ones (32-blocks)
    for e in range(EPB):
        ohcomb[32 * e : 32 * (e + 1), 32 * e : 32 * (e + 1)] = 1.0
    oh4 = np.zeros((128, EPB), dtype=np.float32)     # per-example sum (1/16: group redundancy)
    for e in range(EPB):
        oh4[32 * e : 32 * (e + 1), e] = 1.0 / 16.0
    ones128 = np.ones((128, 1), dtype=np.float32)
    ohcomb_d = nc.inline_tensor(ohcomb, name="ohcomb")
    oh4_d = nc.inline_tensor(oh4, name="oh4")
    ones_d = nc.inline_tensor(ones128, name="ones128")

    with tile.TileContext(nc) as tc:
        _kern(tc, pr_d, gt_d, outv_d, mm_d, ohcomb_d, oh4_d, ones_d,
              f32, mybir, Alu, Ax)
    nc.compile()
    return nc


def _kern(tc, pr_d, gt_d, outv_d, mm_d, ohcomb_d, oh4_d, ones_d,
          f32, mybir, Alu, Ax):
    import concourse.bass as bass  # noqa: F401

    nc = tc.nc
    i16 = mybir.dt.int16
    u16 = mybir.dt.uint16

    from contextlib import ExitStack
    _stack = ExitStack()
    persist = _stack.enter_context(tc.tile_pool(name="persist", bufs=1))
    planes = _stack.enter_context(tc.tile_pool(name="planes", bufs=1))
    scr = _stack.enter_context(tc.tile_pool(name="scr", bufs=1))
    step_p = _stack.enter_context(tc.tile_pool(name="step", bufs=2))
    psum_p = _stack.enter_context(tc.tile_pool(name="psum", bufs=2, space="PSUM"))

    # --- constants to SBUF
    ohcomb = persist.tile([128, 128], f32, tag="ohcomb")
    oh4 = persist.tile([128, EPB], f32, tag="oh4")
    ones128 = persist.tile([128, 1], f32, tag="ones128")
    nc.sync.dma_start(ohcomb[:], ohcomb_d.ap())
    nc.sync.dma_start(oh4[:], oh4_d.ap())
    nc.sync.dma_start(ones128[:], ones_d.ap())

    # --- persistent matching state
    cvS = persist.tile([128, 64], f32, tag="cvS")    # candidate values
    ciSf = persist.tile([128, 64], f32, tag="ciSf")  # candidate row ids (f32)
    ciI = persist.tile([128, 64], i16, tag="ciI")    # candidate row ids (i16)
    mm64 = persist.tile([128, 64], f32, tag="mm64")  # matched-entry mask
    minv = persist.tile([128, 1], f32, tag="minv")   # min selected value (rows 32e valid)
    hm32 = persist.tile([128, 32], f32, tag="hm32")  # col0 = per-partition max
    nc.vector.memset(mm64[:], 0.0)
    nc.vector.memset(minv[:], 1e30)
    nc.vector.memset(hm32[:], 0.0)

    # staging for top-8 extraction
    stagv = persist.tile([128, 8 * 2 * EPB], f32, tag="stagv")
    stagiu = persist.tile([128, 8 * 2 * EPB], u16, tag="stagiu")
    stagif = persist.tile([128, 8 * 2 * EPB], f32, tag="stagif")

    # ---------------- Phase A/B: per-example IoU + top-8 ----------------
    for e in range(EPB):
        # gt halves: [128, 4] each
        gts = []
        for h in range(2):
            g = scr.tile([128, 4], f32, tag="gtile")
            nc.sync.dma_start(g[:], gt_d.ap()[e, 128 * h : 128 * (h + 1), :])
            gts.append(g)
        # pr planes: one broadcast DMA [128, N*4]; per-coord strided views
        plv = planes.tile([128, N * 4], f32, tag="plv")
        psrc = pr_d.ap()[e].rearrange("n c -> (n c)").unsqueeze(0).partition_broadcast(128).squeeze(1)
        nc.sync.dma_start(plv[:], psrc)
        plv3 = plv[:].rearrange("p (n c) -> p n c", c=4)
        px1r, py1r, px2r, py2r = (plv3[:, :, c] for c in range(4))
        # pa_r = (px2-px1)*(py2-py1) replicated
        pa_r = planes.tile([128, N], f32, tag="pa_r")
        t1 = scr.tile([128, N], f32, tag="t1", name="t1a")
        nc.vector.tensor_tensor(out=t1[:], in0=px2r, in1=px1r, op=Alu.subtract)
        nc.vector.tensor_tensor(out=pa_r[:], in0=py2r, in1=py1r, op=Alu.subtract)
        nc.vector.tensor_tensor(out=pa_r[:], in0=pa_r[:], in1=t1[:], op=Alu.mult)

        for h in range(2):
            g = gts[h]
            gx1, gy1, gx2, gy2 = (g[:, c : c + 1] for c in range(4))
            ga = scr.tile([128, 1], f32, tag="ga")
            gw = scr.tile([128, 1], f32, tag="gw")
            nc.vector.tensor_tensor(out=gw[:], in0=gx2, in1=gx1, op=Alu.subtract)
            nc.vector.tensor_tensor(out=ga[:], in0=gy2, in1=gy1, op=Alu.subtract)
            nc.vector.tensor_tensor(out=ga[:], in0=ga[:], in1=gw[:], op=Alu.mult)

            t1 = scr.tile([128, N], f32, tag="t1", name="t1b")
            t2 = scr.tile([128, N], f32, tag="t2")
            # wy = relu(min(py2,gy2) - max(py1,gy1))
            nc.vector.tensor_scalar(out=t1[:], in0=py1r, scalar1=gy1, scalar2=None, op0=Alu.max)
            nc.vector.scalar_tensor_tensor(out=t2[:], in0=py2r, scalar=gy2, in1=t1[:],
                                           op0=Alu.min, op1=Alu.subtract)
            nc.vector.tensor_scalar(out=t2[:], in0=t2[:], scalar1=0.0, scalar2=None, op0=Alu.max)
            # wxr = min(px2,gx2) - max(px1,gx1); inter = relu(wxr)*wy
            nc.vector.tensor_scalar(out=t1[:], in0=px1r, scalar1=gx1, scalar2=None, op0=Alu.max)
            nc.vector.scalar_tensor_tensor(out=t1[:], in0=px2r, scalar=gx2, in1=t1[:],
                                           op0=Alu.min, op1=Alu.subtract)
            nc.vector.scalar_tensor_tensor(out=t1[:], in0=t1[:], scalar=0.0, in1=t2[:],
                                           op0=Alu.max, op1=Alu.mult)
            # union = pa + ga - inter ; iou = inter/union (no eps, as reference _box_iou)
            nc.vector.tensor_scalar(out=t2[:], in0=pa_r[:], scalar1=ga[:], scalar2=None, op0=Alu.add)
            nc.vector.tensor_tensor(out=t2[:], in0=t2[:], in1=t1[:], op=Alu.subtract)
            nc.vector.reciprocal(out=t2[:], in_=t2[:])
            nc.vector.tensor_tensor(out=t1[:], in0=t1[:], in1=t2[:], op=Alu.mult)
            # top-8 per gt column
            sl = slice(8 * (2 * e + h), 8 * (2 * e + h) + 8)
            nc.vector.max(out=stagv[:, sl], in_=t1[:])
            nc.vector.max_index(out=stagiu[:, sl], in_max=stagv[:, sl], in_values=t1[:])

    # ---------------- Phase C: restripe into matching layout ----------------
    nc.vector.tensor_copy(stagif[:], stagiu[:])  # u16 -> f32
    stagii = persist.tile([128, 8 * 2 * EPB], i16, tag="stagii")
    nc.vector.tensor_copy(stagii[:], stagiu[:])  # u16 -> i16
    cvS3 = cvS[:].rearrange("p (s l) -> p s l", l=8)
    ciSf3 = ciSf[:].rearrange("p (s l) -> p s l", l=8)
    ciI3 = ciI[:].rearrange("p (s l) -> p s l", l=8)
    for e in range(EPB):
        for h in range(2):
            g16 = 16 * (2 * e + h)
            sl = slice(8 * (2 * e + h), 8 * (2 * e + h) + 8)
            nc.sync.dma_start(cvS3[g16 : g16 + 16, :, :], stagv[:, sl])
            nc.sync.dma_start(ciSf3[g16 : g16 + 16, :, :], stagif[:, sl])
            nc.sync.dma_start(ciI3[g16 : g16 + 16, :, :], stagii[:, sl])

    # ---------------- Phase D: 256 greedy matching steps ----------------
    hv8b = None
    for t in range(M):
        hv8 = step_p.tile([128, 8], f32, tag="hv8")
        nc.vector.tensor_reduce(out=hv8[:], in_=cvS3, axis=Ax.X, op=Alu.max)
        nc.vector.tensor_reduce(out=hm32[:, 0:1], in_=hv8[:], axis=Ax.X, op=Alu.max)
        zt = step_p.tile([128, 32], f32, tag="zt")
        nc.vector.transpose(out=zt[:], in_=hm32[:])
        gzc = step_p.tile([128, 1], f32, tag="gzc")
        nc.vector.tensor_reduce(out=gzc[:], in_=zt[:], axis=Ax.X, op=Alu.max)
        ez = step_p.tile([128, 32], f32, tag="ez")
        nc.vector.tensor_scalar(out=ez[:], in0=zt[:], scalar1=gzc[:], scalar2=None, op0=Alu.is_equal)
        ezt = step_p.tile([128, 32], f32, tag="ezt")
        nc.vector.transpose(out=ezt[:], in_=ez[:])
        # eqh: per-partition head == partition max (parallel track)
        eqh = step_p.tile([128, 8], f32, tag="eqh")
        nc.vector.tensor_scalar(out=eqh[:], in0=hv8[:], scalar1=hm32[:, 0:1], scalar2=None, op0=Alu.is_equal)
        eqL = step_p.tile([128, 64], f32, tag="eqL")
        hv8b = hv8[:].unsqueeze(2).to_broadcast([128, 8, 8])
        nc.vector.tensor_tensor(out=eqL[:].rearrange("p (s l) -> p s l", l=8), in0=cvS3, in1=hv8b, op=Alu.is_equal)
        # winner (partition, slot)
        eq8 = step_p.tile([128, 8], f32, tag="eq8")
        nc.vector.tensor_scalar(out=eq8[:], in0=eqh[:], scalar1=ezt[:, 0:1], scalar2=None, op0=Alu.mult)
        eq8b = eq8[:].unsqueeze(2).to_broadcast([128, 8, 8])
        em64 = step_p.tile([128, 64], f32, tag="em64")
        nc.vector.tensor_tensor(out=em64[:].rearrange("p (s l) -> p s l", l=8), in0=eqL[:].rearrange("p (s l) -> p s l", l=8), in1=eq8b, op=Alu.mult)
        # i* per example block (sum of em64*ci over block via PE)
        ic = step_p.tile([128, 64], f32, tag="ic")
        nc.vector.tensor_tensor(out=ic[:], in0=em64[:], in1=ciSf[:], op=Alu.mult)
        icr = step_p.tile([128, 1], f32, tag="icr")
        nc.vector.tensor_reduce(out=icr[:], in_=ic[:], axis=Ax.X, op=Alu.add)
        ipb = psum_p.tile([128, 1], f32, tag="ipb")
        nc.tensor.matmul(ipb[:], ohcomb[:], icr[:], start=True, stop=True)
        # kill: all entries with used row + whole winning column
        kill64 = step_p.tile([128, 64], f32, tag="kill64")
        nc.vector.tensor_scalar(out=kill64[:], in0=ciSf[:], scalar1=ipb[:], scalar2=None, op0=Alu.is_equal)
        km = step_p.tile([128, 64], f32, tag="km")
        nc.vector.tensor_tensor(out=km[:].rearrange("p (s l) -> p s l", l=8), in0=kill64[:].rearrange("p (s l) -> p s l", l=8), in1=eq8b, op=Alu.max)
        nc.vector.scalar_tensor_tensor(out=cvS[:], in0=km[:], scalar=NEG_BIG, in1=cvS[:],
                                       op0=Alu.mult, op1=Alu.add)
        # bookkeeping (off critical path)
        nc.vector.tensor_tensor(out=mm64[:], in0=mm64[:], in1=em64[:], op=Alu.max)
        nc.vector.tensor_tensor(out=minv[:], in0=minv[:], in1=gzc[:], op=Alu.min)

    # ---------------- Phase E: loss from matched pairs ----------------
    # matched row per column
    mrowf = persist.tile([128, 8], f32, tag="mrowf")
    mr64 = scr.tile([128, 64], f32, tag="mr64")
    nc.vector.tensor_tensor(out=mr64[:], in0=mm64[:], in1=ciSf[:], op=Alu.mult)
    nc.vector.tensor_reduce(out=mrowf[:], in_=mr64[:].rearrange("p (s l) -> p s l", l=8), axis=Ax.X, op=Alu.add)
    nc.vector.tensor_scalar(out=mrowf[:], in0=mrowf[:], scalar1=float(N - 1), scalar2=0.0,
                            op0=Alu.min, op1=Alu.max)
    # flat element offsets (row*4) as uint16 for indirect_copy
    mrow4 = persist.tile([128, 8], f32, tag="mrow4")
    nc.vector.tensor_scalar(out=mrow4[:], in0=mrowf[:], scalar1=4.0, scalar2=None, op0=Alu.mult)
    mrowi = persist.tile([128, 8], u16, tag="mrowi")
    nc.vector.tensor_copy(mrowi[:], mrow4[:])
    # replicated full boxes per example block: prall [128, N*4], gtall [128, M*4]
    prall = planes.tile([128, N * 4], f32, tag="prall")
    gtall = persist.tile([128, M * 4], f32, tag="gtall")
    for e in range(EPB):
        psrc = pr_d.ap()[e].rearrange("n c -> (n c)").unsqueeze(0).partition_broadcast(32).squeeze(1)
        nc.sync.dma_start(prall[32 * e : 32 * (e + 1), :], psrc)
        gsrc = gt_d.ap()[e].rearrange("n c -> (n c)").unsqueeze(0).partition_broadcast(32).squeeze(1)
        nc.sync.dma_start(gtall[32 * e : 32 * (e + 1), :], gsrc)
    # constant gt-column index tile (wrapped for ap_gather)
    gti_np = np.zeros((128, 8), dtype=np.uint16)
    for p in range(128):
        gq = p // 16
        for s in range(8):
            gti_np[p, s] = ((gq % 2) * 128 + (p % 16) * 8 + s) * 4
    gti_d = nc.inline_tensor(gti_np, name="gtidx")
    gti = persist.tile([128, 8], u16, tag="gti")
    nc.sync.dma_start(gti[:], gti_d.ap())
    # gather matched pr boxes and col gt boxes: [128, 128, 4]
    gpr4 = persist.tile([128, 128 * 4], f32, tag="gpr4")
    gtb4 = persist.tile([128, 128 * 4], f32, tag="gtb4")
    if _DISABLE_GATHER:
        nc.vector.memset(gpr4[:], 0.1)
        nc.vector.memset(gtb4[:], 0.1)
    else:
        nc.gpsimd.indirect_copy(out=gpr4[:].rearrange("p (i c) -> p i c", c=4),
                                data=prall[:].rearrange("p (n c) -> p n c", c=4), idxs=mrowi[:],
                                i_know_ap_gather_is_preferred=True)
        nc.gpsimd.indirect_copy(out=gtb4[:].rearrange("p (i c) -> p i c", c=4),
                                data=gtall[:].rearrange("p (n c) -> p n c", c=4), idxs=gti[:],
                                i_know_ap_gather_is_preferred=True)
    gco = [gpr4[:].rearrange("p (i c) -> p i c", c=4)[:, :, cc] for cc in range(4)]
    gtW = [gtb4[:].rearrange("p (i c) -> p i c", c=4)[:, :, cc] for cc in range(4)]
    g1x, g1y, g2x, g2y = gco
    w1x, w1y, w2x, w2y = gtW

    def tt(out, a, b, op):
        nc.vector.tensor_tensor(out=out, in0=a, in1=b, op=op)

    t1 = scr.tile([128, 128], f32, tag="p1")
    t2 = scr.tile([128, 128], f32, tag="p2")
    t3 = scr.tile([128, 128], f32, tag="p3")
    u1 = scr.tile([128, 128], f32, tag="p4")
    u2 = scr.tile([128, 128], f32, tag="p5")
    a1 = scr.tile([128, 128], f32, tag="p6")
    pos = scr.tile([128, 128], f32, tag="pos")
    neg = scr.tile([128, 128], f32, tag="neg")
    tt(t1[:], g1x, w1x, Alu.max)         # xi1
    tt(t2[:], g2x, w2x, Alu.min)         # xi2
    tt(t2[:], t2[:], t1[:], Alu.subtract)
    nc.vector.tensor_scalar(out=t2[:], in0=t2[:], scalar1=0.0, scalar2=None, op0=Alu.max)  # wx
    tt(t1[:], g1y, w1y, Alu.max)
    tt(t3[:], g2y, w2y, Alu.min)
    tt(t3[:], t3[:], t1[:], Alu.subtract)
    nc.vector.tensor_scalar(out=t3[:], in0=t3[:], scalar1=0.0, scalar2=None, op0=Alu.max)  # wy
    tt(t3[:], t2[:], t3[:], Alu.mult)          # inter
    tt(t1[:], g2x, g1x, Alu.subtract)
    tt(t2[:], g2y, g1y, Alu.subtract)
    tt(a1[:], t1[:], t2[:], Alu.mult)          # a1
    tt(t1[:], w2x, w1x, Alu.subtract)
    tt(t2[:], w2y, w1y, Alu.subtract)
    tt(t1[:], t1[:], t2[:], Alu.mult)          # a2
    tt(t2[:], a1[:], t1[:], Alu.add)
    tt(t2[:], t2[:], t3[:], Alu.subtract)      # union
    nc.vector.tensor_scalar(out=u2[:], in0=t2[:], scalar1=EPS, scalar2=None, op0=Alu.add)
    nc.vector.reciprocal(out=u2[:], in_=u2[:])
    tt(u2[:], t3[:], u2[:], Alu.mult)          # iou
    tt(t3[:], g1x, w1x, Alu.min)
    tt(u1[:], g2x, w2x, Alu.max)
    tt(t3[:], u1[:], t3[:], Alu.subtract)      # xc2-xc1
    tt(u1[:], g1y, w1y, Alu.min)
    tt(t1[:], g2y, w2y, Alu.max)
    tt(u1[:], t1[:], u1[:], Alu.subtract)      # yc2-yc1
    tt(t3[:], t3[:], u1[:], Alu.mult)          # areac
    tt(u1[:], t3[:], t2[:], Alu.subtract)      # areac - union
    nc.vector.tensor_scalar(out=t3[:], in0=t3[:], scalar1=EPS, scalar2=None, op0=Alu.add)
    nc.vector.reciprocal(out=t3[:], in_=t3[:])
    tt(u1[:], u1[:], t3[:], Alu.mult)          # penalty
    nc.vector.tensor_scalar(out=u2[:], in0=u2[:], scalar1=-1.0, scalar2=1.0, op0=Alu.mult, op1=Alu.add)  # 1-iou
    tt(pos[:], u2[:], u1[:], Alu.add)          # posterm
    # negterm for matched rows: 1 + (x2*y2 - a1)/(x2*y2 + eps)
    tt(t1[:], g2x, g2y, Alu.mult)
    tt(t2[:], t1[:], a1[:], Alu.subtract)
    nc.vector.tensor_scalar(out=t1[:], in0=t1[:], scalar1=EPS, scalar2=None, op0=Alu.add)
    nc.vector.reciprocal(out=t1[:], in_=t1[:])
    tt(t2[:], t2[:], t1[:], Alu.mult)
    nc.vector.tensor_scalar(out=neg[:], in0=t2[:], scalar1=1.0, scalar2=None, op0=Alu.add)

    red2 = persist.tile([128, 2], f32, tag="red2")
    nc.vector.tensor_reduce(out=red2[:, 0:1], in_=pos[:], axis=Ax.X, op=Alu.add)
    nc.vector.tensor_reduce(out=red2[:, 1:2], in_=neg[:], axis=Ax.X, op=Alu.add)
    psum1 = psum_p.tile([EPB, 2], f32, tag="psum1")
    nc.tensor.matmul(psum1[:], oh4[:], red2[:], start=True, stop=True)

    # negfull per example
    red4 = persist.tile([128, EPB], f32, tag="red4")
    for e in range(EPB):
        prt = scr.tile([128, 32, 4], f32, tag="prt")
        nc.sync.dma_start(prt[:], pr_d.ap()[e].rearrange("(p k) c -> p k c", p=128))
        x1, y1, x2, y2 = (prt[:, :, c] for c in range(4))
        n1 = scr.tile([128, 32], f32, tag="n1")
        n2 = scr.tile([128, 32], f32, tag="n2")
        n3 = scr.tile([128, 32], f32, tag="n3")
        tt(n1[:], x2, x1, Alu.subtract)
        tt(n2[:], y2, y1, Alu.subtract)
        tt(n1[:], n1[:], n2[:], Alu.mult)      # a1
        tt(n2[:], x2, y2, Alu.mult)            # t = x2*y2
        tt(n3[:], n2[:], n1[:], Alu.subtract)
        nc.vector.tensor_scalar(out=n2[:], in0=n2[:], scalar1=EPS, scalar2=None, op0=Alu.add)
        nc.vector.reciprocal(out=n2[:], in_=n2[:])
        tt(n3[:], n3[:], n2[:], Alu.mult)
        nc.vector.tensor_scalar(out=n3[:], in0=n3[:], scalar1=1.0, scalar2=None, op0=Alu.add)
        nc.vector.tensor_reduce(out=red4[:, e : e + 1], in_=n3[:], axis=Ax.X, op=Alu.add)
    psum2 = psum_p.tile([1, EPB], f32, tag="psum2")
    nc.tensor.matmul(psum2[:], ones128[:], red4[:], start=True, stop=True)

    # assemble output [EPB, 8]
    out_sb = persist.tile([EPB, 8], f32, tag="out_sb")
    nc.vector.memset(out_sb[:], 0.0)
    nc.vector.tensor_copy(out_sb[:, 0:2], psum1[:])
    # transpose psum2 [1, EPB] -> [EPB, 1] via 32x32 stream transpose
    tr_in = persist.tile([32, 32], f32, tag="tr_in")
    tr_out = persist.tile([32, 32], f32, tag="tr_out")
    nc.vector.memset(tr_in[:], 0.0)
    nc.vector.tensor_copy(tr_in[0:1, 0:EPB], psum2[:])
    nc.vector.transpose(out=tr_out[:], in_=tr_in[:])
    nc.vector.tensor_copy(out_sb[:, 2:3], tr_out[0:EPB, 0:1])
    nc.sync.dma_start(outv_d.ap()[:, 3:4], minv[0:128:32, 0:1])
    nc.sync.dma_start(outv_d.ap()[:, 0:3], out_sb[:, 0:3])
    nc.sync.dma_start(mm_d.ap(), mm64[:])

    _stack.close()


# ---------------------------------------------------------------------------
# Host-side fallback (exact device-algorithm replica; never fires in practice)
# ---------------------------------------------------------------------------

def _loss_example_numpy(pr, gt):
    pr = pr.astype(np.float64)
    gt = gt.astype(np.float64)
    lt = np.maximum(pr[:, None, :2], gt[None, :, :2])
    rb = np.minimum(pr[:, None, 2:], gt[None, :, 2:])
    wh = np.clip(rb - lt, 0.0, None)
    inter = wh[..., 0] * wh[..., 1]
    area_p = (pr[:, 2] - pr[:, 0]) * (pr[:, 3] - pr[:, 1])
    area_g = (gt[:, 2] - gt[:, 0]) * (gt[:, 3] - gt[:, 1])
    union = area_p[:, None] + area_g[None, :] - inter
    iou = inter / union
    m = iou.copy()
    pr_idx = np.zeros(M, dtype=np.int64)
    gt_order = np.zeros(M, dtype=np.int64)
    for t in range(M):
        idx = np.argmax(m)
        i, j = idx // M, idx % M
        pr_idx[t], gt_order[t] = i, j
        m[i, :] = -np.inf
        m[:, j] = -np.inf

    def giou_mean(b1, b2):
        xi1 = np.maximum(b1[:, 0], b2[:, 0]); yi1 = np.maximum(b1[:, 1], b2[:, 1])
        xi2 = np.minimum(b1[:, 2], b2[:, 2]); yi2 = np.minimum(b1[:, 3], b2[:, 3])
        inter = np.clip(xi2 - xi1, 0, None) * np.clip(yi2 - yi1, 0, None)
        a1 = (b1[:, 2] - b1[:, 0]) * (b1[:, 3] - b1[:, 1])
        a2 = (b2[:, 2] - b2[:, 0]) * (b2[:, 3] - b2[:, 1])
        un = a1 + a2 - inter
        iou = inter / (un + EPS)
        xc1 = np.minimum(b1[:, 0], b2[:, 0]); yc1 = np.minimum(b1[:, 1], b2[:, 1])
        xc2 = np.maximum(b1[:, 2], b2[:, 2]); yc2 = np.maximum(b1[:, 3], b2[:, 3])
        ac = (xc2 - xc1) * (yc2 - yc1)
        giou = iou - (ac - un) / (ac + EPS)
        return np.mean(1.0 - giou)

    mask = np.zeros(N, dtype=bool)
    mask[pr_idx] = True
    neg_idx = np.nonzero(~mask)[0][: N - M]
    l1 = giou_mean(pr[pr_idx], gt[gt_order])
    l2 = giou_mean(pr[neg_idx], np.zeros((N - M, 4)))
    return l1 + l2


# ---------------------------------------------------------------------------
# Entry point
# ---------------------------------------------------------------------------

def kernel(pr_boxes, gt_boxes, pr_class_logits=None, gt_classes=None, **_unused):
    from concourse.bass_utils import run_bass_kernel_spmd

    pr_boxes = np.ascontiguousarray(pr_boxes, dtype=np.float32)
    gt_boxes = np.ascontiguousarray(gt_boxes, dtype=np.float32)

    if "nc" not in _CACHE:
        _CACHE["nc"] = _build_program()
    nc = _CACHE["nc"]

    in_maps = []
    for k in range(NCORES):
        in_maps.append({
            "pr_boxes": pr_boxes[EPB * k : EPB * (k + 1)],
            "gt_boxes": gt_boxes[EPB * k : EPB * (k + 1)],
        })
    res = run_bass_kernel_spmd(nc, in_maps, core_ids=list(range(NCORES)))

    total = np.float32(0.0)
    for k in range(NCORES):
        outv = res.results[k]["outv"]  # [EPB, 8]
        for e in range(EPB):
            possum, negm, negfull, mv = outv[e, 0], outv[e, 1], outv[e, 2], outv[e, 3]
            if mv <= 0.0:
                le = np.float32(_loss_example_numpy(
                    pr_boxes[EPB * k + e], gt_boxes[EPB * k + e]))
            else:
                le = np.float32(possum / M + (negfull - negm) / (N - M))
            total = np.float32(total + le)
    return np.float32(total / B / 2.0)

